# revision 14
# baseline (speedup 1.0000x reference)
"""DisMax loss first part: logits = -(|s|*d + mean_c(|s|*d)) / temp, where
d[b,c] = ||fn_b - pn_c|| / sqrt(2) = sqrt(1 - cos(f_b, p_c)) for l2-normalized rows.

Strategy: data-parallel over the batch across 8 NeuronCores. Each core:
  [1024, 512] features x [512, 10000] transposed prototypes -> [1024, 10000].
Both operands arrive host-transposed and host-cast to bf16 (layout/precision
prep only); all numerics run on device:
  - feature row norms: DVE square+accum on the batch-major copy, one ACT
    abs-rsqrt -> invf [128, 8]; features themselves stay RAW (negated on
    host) and invf folds into the main ACT as its per-partition scale;
  - prototype class norms: DVE/GPSIMD squares -> PE ones-matmul column
    sums -> chunked ACT abs-rsqrt (bf16 row) -> PE partition-broadcast ->
    in-place DVE normalize of the persistent bf16 pn operand;
  - main GEMM: bf16, fp32 PSUM, [128 x 500] chunks into 4-bank groups; ACT
    computes dist = sqrt(1 + G*invf) straight out of PSUM (G = -f.pn) with
    fused row-sum accumulation, writing bf16;
  - DVE applies out = dist*c0 + rowsum*c1 (c0 = -|scale|/temp,
    c1 = c0/10000) at 4x bf16 rate; 1.25 MB DMAs stream the bf16 result;
  - host upcasts the bf16 logits to f32 (within the 2e-2 tolerance).
"""

import sys
import types

for _p in ("/opt/trn_rl_repo", "/root/.axon_site"):
    if _p not in sys.path:
        sys.path.insert(0, _p)

# The NTFF profiling hook module is absent from this image's antenv package;
# inject the ctypes-based equivalent so trace=True works when requested.
if "antenv.axon_hooks" not in sys.modules:
    try:
        import trn_agent_boot.trn_boot as _tb

        _hook = _tb._ntff_profile_via_ctypes("/opt/axon/libaxon_pjrt.so")
        _m = types.ModuleType("antenv.axon_hooks")
        _m.get_axon_ntff_profile_hook = lambda: _hook
        sys.modules["antenv.axon_hooks"] = _m
    except Exception:
        pass

import ml_dtypes
import numpy as np

import concourse.bacc as bacc
import concourse.tile as tile
import concourse.mybir as mybir
from concourse.bass_utils import run_bass_kernel_spmd

F32 = mybir.dt.float32
BF16 = mybir.dt.bfloat16
ALU = mybir.AluOpType
ACTF = mybir.ActivationFunctionType

N_CORES = 8
B, C, D = 8192, 10000, 512
BPC = B // N_CORES          # 1024 batch rows per core
NB = BPC // 128             # 8 batch tiles
ND = D // 128               # 4 contraction tiles
PCH = 1000                  # prototype-prep chunk (columns)
NPCH = C // PCH             # 10
G1 = 1000                   # main-loop ACT group (2 psum banks)
NG = C // G1                # 10
OUT = 2500                  # output store chunk (640 KB bf16)
NOUT = C // OUT             # 4
NI = 3                      # batch tiles interleaved with prototype prep


def build_nc():
    nc = bacc.Bacc("TRN2", target_bir_lowering=False, debug=False,
                   num_devices=N_CORES)
    ft_h = nc.dram_tensor("ft", [D, BPC], BF16, kind="ExternalInput")
    fb_h = nc.dram_tensor("fb", [BPC, D], BF16, kind="ExternalInput")
    pt_h = nc.dram_tensor("pt", [D, C], BF16, kind="ExternalInput")
    s_h = nc.dram_tensor("s", [1, 2], F32, kind="ExternalInput")
    o_h = nc.dram_tensor("o", [BPC, C], BF16, kind="ExternalOutput")

    from contextlib import ExitStack

    with tile.TileContext(nc) as tc:
        with ExitStack() as stack:
            ep = stack.enter_context
            const_pool = ep(tc.tile_pool(name="const", bufs=1))
            persist_pool = ep(tc.tile_pool(name="persist", bufs=1))
            fst_pool = ep(tc.tile_pool(name="fst", bufs=1))
            sq_pool = ep(tc.tile_pool(name="sq", bufs=2))
            dq_pool = ep(tc.tile_pool(name="dq", bufs=NI))
            ob_pool = ep(tc.tile_pool(name="ob", bufs=3))
            small_pool = ep(tc.tile_pool(name="small", bufs=2))
            ps_c = ep(tc.tile_pool(name="ps_c", bufs=4, space="PSUM"))

            # persistent operands (pn is chunk-major: [p, chunk, d, col])
            pn = persist_pool.tile([128, NPCH, ND, PCH], BF16, tag="pn")
            fT = persist_pool.tile([128, ND, BPC], BF16, tag="fT")    # 8 KB/p
            invpb = persist_pool.tile([128, C], BF16, tag="invpb")    # 20 KB/p
            invf = persist_pool.tile([128, NB], F32, tag="invf")
            cb = persist_pool.tile([128, 2], F32, tag="cb")           # c0, c1

            ones_b = const_pool.tile([128, 1], BF16, tag="ones_b")
            nc.vector.memset(ones_b[:, :], 1.0)
            ones_r = const_pool.tile([1, 128], BF16, tag="ones_r")
            nc.vector.memset(ones_r[:, :], 1.0)
            ones_rf = const_pool.tile([1, 128], F32, tag="ones_rf")
            nc.vector.memset(ones_rf[:, :], 1.0)

            # ---- input DMAs (front-loaded) --------------------------------
            stile = const_pool.tile([1, 2], F32, tag="stile")
            nc.sync.dma_start(stile[:, :], s_h[:, :])
            ft_r = ft_h[:, :].rearrange("(t p) b -> p t b", p=128)
            nc.sync.dma_start(fT[:, :, :], ft_r)
            fb_r = fb_h[:, :].rearrange("(t p) d -> p t d", p=128)
            fst = fst_pool.tile([128, NB, D], BF16, tag="fst")
            nc.sync.dma_start(fst[:, :, :], fb_r)
            pt_r = pt_h[:, :].rearrange("(t p) c -> p t c", p=128)
            for c in range(NPCH):
                nc.sync.dma_start(pn[:, c, :, :],
                                  pt_r[:, :, c * PCH:(c + 1) * PCH])

            # ---- scalar params: c0 = -|ds|/temp, c1 = c0/C ----------------
            cv = const_pool.tile([1, 2], F32, tag="cvals")
            tmp = const_pool.tile([1, 2], F32, tag="scaltmp")
            nc.scalar.activation(tmp[:, 0:1], stile[:, 0:1], ACTF.Abs)
            nc.vector.reciprocal(tmp[:, 1:2], stile[:, 1:2])
            nc.vector.scalar_tensor_tensor(cv[:, 0:1], tmp[:, 0:1], -1.0,
                                           tmp[:, 1:2], op0=ALU.mult,
                                           op1=ALU.mult)
            nc.vector.tensor_scalar(cv[:, 1:2], cv[:, 0:1], 1.0 / C, None,
                                    op0=ALU.mult)
            ps_s = ps_c.tile([128, 2, 512], F32, tag="pc", name="ps_s")
            nc.tensor.matmul(ps_s[:, 0, :2], ones_rf[:, :], cv[:, :],
                             start=True, stop=True)
            nc.vector.tensor_copy(cb[:, :], ps_s[:, 0, :2])

            # ---- feature norms --------------------------------------------
            fss = small_pool.tile([128, NB], F32, tag="fss", bufs=1)
            for t in range(NB):
                fsq = sq_pool.tile([128, ND, PCH], BF16, tag="sq",
                                   name=f"fsq_{t}")
                fsqv = fsq[:, :, :].rearrange("p a b -> p (a b)")
                nc.vector.scalar_tensor_tensor(
                    fsqv[:, :D], fst[:, t, :], 1.0, fst[:, t, :],
                    op0=ALU.mult, op1=ALU.mult,
                    accum_out=fss[:, t:t + 1])
            nc.scalar.activation(invf[:, :], fss[:, :],
                                 ACTF.Abs_reciprocal_sqrt)

            # ---- main-loop building blocks --------------------------------
            def gemm_group(i, g, dq, rs):
                pc = ps_c.tile([128, 2, 512], F32, tag="pc",
                               name=f"pc_{i}_{g}")
                for d in range(ND):
                    for h in range(2):
                        nc.tensor.matmul(
                            pc[:, h, :500],
                            fT[:, d, i * 128:(i + 1) * 128],
                            pn[:, g, d, h * 500:(h + 1) * 500],
                            start=(d == 0), stop=(d == ND - 1))
                # dist = sqrt(1 + G*invf); fused row-chunk sum
                dv = dq[:, g * G1:(g + 1) * G1].rearrange(
                    "p (h x) -> p h x", h=2)
                nc.scalar.activation(
                    dv, pc[:, :, :500], ACTF.Sqrt,
                    bias=1.0, scale=invf[:, i:i + 1],
                    accum_out=rs[:, g:g + 1])

            def finish_tile(i, dq, rs):
                rsum = small_pool.tile([128, 1], F32, tag="rsum", bufs=NI,
                                       name=f"rsum_{i}")
                bvec = small_pool.tile([128, 1], F32, tag="bvec", bufs=NI,
                                       name=f"bvec_{i}")
                nc.vector.reduce_sum(rsum[:, :], rs[:, :],
                                     axis=mybir.AxisListType.X)
                nc.vector.tensor_scalar(bvec[:, :], rsum[:, :], cb[:, 1:2],
                                        None, op0=ALU.mult)
                for q in range(NOUT):
                    ob = ob_pool.tile([128, OUT], BF16, tag="ob",
                                      name=f"ob_{i}_{q}")
                    nc.vector.tensor_scalar(ob[:, :],
                                            dq[:, q * OUT:(q + 1) * OUT],
                                            cb[:, 0:1], bvec[:, 0:1],
                                            op0=ALU.mult, op1=ALU.add)
                    nc.sync.dma_start(
                        o_h[i * 128:(i + 1) * 128, q * OUT:(q + 1) * OUT],
                        ob[:, :])

            # ---- prototype prep interleaved with the first NI batch tiles --
            # chunk c: squares (DVE) -> column-sum matmuls (PE) -> abs-rsqrt
            # (ACT) -> broadcast (PE) -> copy (ACT) -> in-place normalize
            # (DVE); then tiles 0..NI-1 GEMM chunk c immediately.
            dqs = [dq_pool.tile([128, C], BF16, tag="dq", name=f"dq_{i}")
                   for i in range(NI)]
            rss = [small_pool.tile([128, NG], F32, tag="rs", bufs=NI,
                                   name=f"rs_{i}") for i in range(NI)]
            for c in range(NPCH):
                c0, c1 = c * PCH, (c + 1) * PCH
                sq = sq_pool.tile([128, ND * PCH], BF16, tag="sq",
                                  name=f"sq_{c}")
                pflat = pn[:, c, :, :].rearrange("p a b -> p (a b)")
                nc.vector.tensor_tensor(sq[:, :], pflat, pflat, op=ALU.mult)
                # column sums via ones-matmul into a 2-bank tile
                psq = ps_c.tile([128, 2, 512], F32, tag="pc",
                                name=f"psq_{c}")
                sqv = sq[:, :].rearrange("p (a b) -> p a b", a=ND)
                for h in range(2):
                    for d in range(ND):
                        nc.tensor.matmul(
                            psq[0:1, h, :500], ones_b[:, :],
                            sqv[:, d, h * 500:(h + 1) * 500],
                            start=(d == 0), stop=(d == ND - 1))
                # 1/||p|| straight into partition 0 of the broadcast buffer
                nc.scalar.activation(
                    invpb[0:1, c0:c1].rearrange("p (h x) -> p h x", h=2),
                    psq[0:1, :, :500], ACTF.Abs_reciprocal_sqrt)
                # broadcast to all 128 partitions (bf16 matmul) + copy out
                psb = ps_c.tile([128, 2, 512], F32, tag="pc",
                                name=f"psb_{c}")
                for h in range(2):
                    q0 = c0 + h * 500
                    nc.tensor.matmul(psb[:, h, :500], ones_r[:, :],
                                     invpb[0:1, q0:q0 + 500],
                                     start=True, stop=True)
                nc.scalar.copy(
                    invpb[:, c0:c1].rearrange("p (h x) -> p h x", h=2),
                    psb[:, :, :500])
                # in-place normalize: pn = pn * invp  (bf16, per d-tile)
                for d in range(ND):
                    nc.vector.tensor_tensor(pn[:, c, d, :],
                                            pn[:, c, d, :],
                                            invpb[:, c0:c1], op=ALU.mult)
                for i in range(NI):
                    gemm_group(i, c, dqs[i], rss[i])
            for i in range(NI):
                finish_tile(i, dqs[i], rss[i])

            # ---- remaining batch tiles -------------------------------------
            for i in range(NI, NB):
                rs = small_pool.tile([128, NG], F32, tag="rs", bufs=NI,
                                     name=f"rs_{i}")
                dq = dq_pool.tile([128, C], BF16, tag="dq", name=f"dq_{i}")
                for g in range(NG):
                    gemm_group(i, g, dq, rs)
                finish_tile(i, dq, rs)

    nc.compile()
    return nc


_CACHE = {}


def _get_nc():
    if "nc" not in _CACHE:
        _CACHE["nc"] = build_nc()
    return _CACHE["nc"]


def make_in_maps(features, prototypes, distance_scale, temperature):
    f = np.asarray(features, dtype=np.float32)
    # negated so ACT's positive per-partition scale yields 1 - cos
    fneg = (-f).astype(ml_dtypes.bfloat16)
    pt = np.ascontiguousarray(
        np.asarray(prototypes, dtype=np.float32).T).astype(ml_dtypes.bfloat16)
    s = np.array([[np.float32(np.asarray(distance_scale).reshape(-1)[0]),
                   np.float32(np.asarray(temperature).reshape(-1)[0])]],
                 dtype=np.float32)
    in_maps = []
    for i in range(N_CORES):
        fi = fneg[i * BPC:(i + 1) * BPC]
        in_maps.append({
            "ft": np.ascontiguousarray(fi.T),
            "fb": np.ascontiguousarray(fi),
            "pt": pt,
            "s": s,
        })
    return in_maps


def run(features, prototypes, distance_scale, temperature, **kwargs):
    nc = _get_nc()
    in_maps = make_in_maps(features, prototypes, distance_scale, temperature)
    res = run_bass_kernel_spmd(nc, in_maps, core_ids=list(range(N_CORES)),
                               **kwargs)
    out = np.concatenate(
        [np.asarray(res.results[i]["o"]) for i in range(N_CORES)],
        axis=0).astype(np.float32)
    return out, res


def kernel(features, prototypes, distance_scale, temperature):
    out, _ = run(features, prototypes, distance_scale, temperature)
    return out


# revision 16
# speedup vs baseline: 1.1352x; 1.1352x over previous
"""DisMax loss first part: logits = -(|s|*d + mean_c(|s|*d)) / temp, where
d[b,c] = ||fn_b - pn_c|| / sqrt(2) = sqrt(1 - cos(f_b, p_c)) for l2-normalized rows.

Strategy: data-parallel over the batch across 8 NeuronCores. Each core:
  [1024, 512] features x [512, 10000] transposed prototypes -> [1024, 10000].
Both operands arrive host-transposed and host-cast to bf16 (layout/precision
prep only); all numerics run on device:
  - feature row norms: DVE square+accum on the batch-major copy, one ACT
    abs-rsqrt -> invf [128, 8]; features themselves stay RAW (negated on
    host) and invf folds into the main ACT as its per-partition scale;
  - prototype class norms: DVE/GPSIMD squares -> PE ones-matmul column
    sums -> chunked ACT abs-rsqrt (bf16 row) -> PE partition-broadcast ->
    in-place DVE normalize of the persistent bf16 pn operand;
  - main GEMM: bf16, fp32 PSUM, [128 x 500] chunks into 4-bank groups; ACT
    computes dist = sqrt(1 + G*invf) straight out of PSUM (G = -f.pn) with
    fused row-sum accumulation, writing bf16;
  - DVE applies out = dist*c0 + rowsum*c1 (c0 = -|scale|/temp,
    c1 = c0/10000) at 4x bf16 rate; 1.25 MB DMAs stream the bf16 result;
  - host upcasts the bf16 logits to f32 (within the 2e-2 tolerance).
"""

import sys
import types

for _p in ("/opt/trn_rl_repo", "/root/.axon_site"):
    if _p not in sys.path:
        sys.path.insert(0, _p)

# The NTFF profiling hook module is absent from this image's antenv package;
# inject the ctypes-based equivalent so trace=True works when requested.
if "antenv.axon_hooks" not in sys.modules:
    try:
        import trn_agent_boot.trn_boot as _tb

        _hook = _tb._ntff_profile_via_ctypes("/opt/axon/libaxon_pjrt.so")
        _m = types.ModuleType("antenv.axon_hooks")
        _m.get_axon_ntff_profile_hook = lambda: _hook
        sys.modules["antenv.axon_hooks"] = _m
    except Exception:
        pass

import ml_dtypes
import numpy as np

import concourse.bacc as bacc
import concourse.tile as tile
import concourse.mybir as mybir
from concourse.bass_utils import run_bass_kernel_spmd

F32 = mybir.dt.float32
BF16 = mybir.dt.bfloat16
ALU = mybir.AluOpType
ACTF = mybir.ActivationFunctionType

N_CORES = 8
B, C, D = 8192, 10000, 512
BPC = B // N_CORES          # 1024 batch rows per core
NB = BPC // 128             # 8 batch tiles
ND = D // 128               # 4 contraction tiles
PCH = 1000                  # prototype-prep chunk (columns)
NPCH = C // PCH             # 10
G1 = 1000                   # main-loop ACT group (2 psum banks)
NG = C // G1                # 10
OUT = 2500                  # output store chunk (640 KB bf16)
NOUT = C // OUT             # 4
NI = 3                      # batch tiles interleaved with prototype prep


def build_nc():
    nc = bacc.Bacc("TRN2", target_bir_lowering=False, debug=False,
                   num_devices=N_CORES)
    ft_h = nc.dram_tensor("ft", [D, BPC], BF16, kind="ExternalInput")
    fb_h = nc.dram_tensor("fb", [BPC, D], BF16, kind="ExternalInput")
    pt_h = nc.dram_tensor("pt", [D, C], BF16, kind="ExternalInput")
    s_h = nc.dram_tensor("s", [1, 2], F32, kind="ExternalInput")
    o_h = nc.dram_tensor("o", [BPC, C], BF16, kind="ExternalOutput")

    from contextlib import ExitStack

    with tile.TileContext(nc) as tc:
        with ExitStack() as stack:
            ep = stack.enter_context
            const_pool = ep(tc.tile_pool(name="const", bufs=1))
            persist_pool = ep(tc.tile_pool(name="persist", bufs=1))
            fst_pool = ep(tc.tile_pool(name="fst", bufs=1))
            sq_pool = ep(tc.tile_pool(name="sq", bufs=2))
            dq_pool = ep(tc.tile_pool(name="dq", bufs=NI))
            ob_pool = ep(tc.tile_pool(name="ob", bufs=3))
            small_pool = ep(tc.tile_pool(name="small", bufs=2))
            ps_c = ep(tc.tile_pool(name="ps_c", bufs=4, space="PSUM"))

            # persistent operands (pn is chunk-major: [p, chunk, d, col])
            pn = persist_pool.tile([128, NPCH, ND, PCH], BF16, tag="pn")
            fT = persist_pool.tile([128, ND, BPC], BF16, tag="fT")    # 8 KB/p
            invpb = persist_pool.tile([128, C], BF16, tag="invpb")    # 20 KB/p
            invf = persist_pool.tile([128, NB], F32, tag="invf")
            cb = persist_pool.tile([128, 2], F32, tag="cb")           # c0, c1

            ones_b = const_pool.tile([128, 1], BF16, tag="ones_b")
            nc.vector.memset(ones_b[:, :], 1.0)
            ones_r = const_pool.tile([1, 128], BF16, tag="ones_r")
            nc.vector.memset(ones_r[:, :], 1.0)
            ones_rf = const_pool.tile([1, 128], F32, tag="ones_rf")
            nc.vector.memset(ones_rf[:, :], 1.0)

            # ---- input DMAs (front-loaded) --------------------------------
            stile = const_pool.tile([1, 2], F32, tag="stile")
            nc.sync.dma_start(stile[:, :], s_h[:, :])
            ft_r = ft_h[:, :].rearrange("(t p) b -> p t b", p=128)
            nc.sync.dma_start(fT[:, :, :], ft_r)
            fb_r = fb_h[:, :].rearrange("(t p) d -> p t d", p=128)
            fst = fst_pool.tile([128, NB, D], BF16, tag="fst")
            nc.sync.dma_start(fst[:, :, :], fb_r)
            pt_r = pt_h[:, :].rearrange("(t p) c -> p t c", p=128)
            for c in range(NPCH):
                nc.sync.dma_start(pn[:, c, :, :],
                                  pt_r[:, :, c * PCH:(c + 1) * PCH])

            # ---- scalar params: c0 = -|ds|/temp, c1 = c0/C ----------------
            cv = const_pool.tile([1, 2], F32, tag="cvals")
            tmp = const_pool.tile([1, 2], F32, tag="scaltmp")
            nc.scalar.activation(tmp[:, 0:1], stile[:, 0:1], ACTF.Abs)
            nc.vector.reciprocal(tmp[:, 1:2], stile[:, 1:2])
            nc.vector.scalar_tensor_tensor(cv[:, 0:1], tmp[:, 0:1], -1.0,
                                           tmp[:, 1:2], op0=ALU.mult,
                                           op1=ALU.mult)
            nc.vector.tensor_scalar(cv[:, 1:2], cv[:, 0:1], 1.0 / C, None,
                                    op0=ALU.mult)
            ps_s = ps_c.tile([128, 2, 512], F32, tag="pc", name="ps_s")
            nc.tensor.matmul(ps_s[:, 0, :2], ones_rf[:, :], cv[:, :],
                             start=True, stop=True)
            nc.vector.tensor_copy(cb[:, :], ps_s[:, 0, :2])

            # ---- feature norms --------------------------------------------
            fss = small_pool.tile([128, NB], F32, tag="fss", bufs=1)
            for t in range(NB):
                fsq = sq_pool.tile([128, ND, PCH], BF16, tag="sq",
                                   name=f"fsq_{t}")
                fsqv = fsq[:, :, :].rearrange("p a b -> p (a b)")
                nc.vector.scalar_tensor_tensor(
                    fsqv[:, :D], fst[:, t, :], 1.0, fst[:, t, :],
                    op0=ALU.mult, op1=ALU.mult,
                    accum_out=fss[:, t:t + 1])
            nc.scalar.activation(invf[:, :], fss[:, :],
                                 ACTF.Abs_reciprocal_sqrt)

            # ---- main-loop building blocks --------------------------------
            def gemm_group(i, g, dq, rs):
                pc = ps_c.tile([128, 2, 512], F32, tag="pc",
                               name=f"pc_{i}_{g}")
                for d in range(ND):
                    for h in range(2):
                        nc.tensor.matmul(
                            pc[:, h, :500],
                            fT[:, d, i * 128:(i + 1) * 128],
                            pn[:, g, d, h * 500:(h + 1) * 500],
                            start=(d == 0), stop=(d == ND - 1))
                # dist = sqrt(1 + G*invf); fused row-chunk sum
                dv = dq[:, g * G1:(g + 1) * G1].rearrange(
                    "p (h x) -> p h x", h=2)
                nc.scalar.activation(
                    dv, pc[:, :, :500], ACTF.Sqrt,
                    bias=1.0, scale=invf[:, i:i + 1],
                    accum_out=rs[:, g:g + 1])

            def finish_tile(i, dq, rs):
                rsum = small_pool.tile([128, 1], F32, tag="rsum", bufs=NI,
                                       name=f"rsum_{i}")
                bvec = small_pool.tile([128, 1], F32, tag="bvec", bufs=NI,
                                       name=f"bvec_{i}")
                nc.vector.reduce_sum(rsum[:, :], rs[:, :],
                                     axis=mybir.AxisListType.X)
                nc.vector.tensor_scalar(bvec[:, :], rsum[:, :], cb[:, 1:2],
                                        None, op0=ALU.mult)
                for q in range(NOUT):
                    ob = ob_pool.tile([128, OUT], BF16, tag="ob",
                                      name=f"ob_{i}_{q}")
                    nc.vector.tensor_scalar(ob[:, :],
                                            dq[:, q * OUT:(q + 1) * OUT],
                                            cb[:, 0:1], bvec[:, 0:1],
                                            op0=ALU.mult, op1=ALU.add)
                    nc.sync.dma_start(
                        o_h[i * 128:(i + 1) * 128, q * OUT:(q + 1) * OUT],
                        ob[:, :])

            # ---- prototype prep ------------------------------------------
            # chunk c: squares (DVE) -> column-sum matmuls (PE) -> abs-rsqrt
            # (ACT) -> broadcast (PE) -> copy (ACT) -> in-place normalize
            # (DVE); the main loop's early tiles overlap via chunk deps.
            for c in range(NPCH):
                c0, c1 = c * PCH, (c + 1) * PCH
                sq = sq_pool.tile([128, ND * PCH], BF16, tag="sq",
                                  name=f"sq_{c}")
                pflat = pn[:, c, :, :].rearrange("p a b -> p (a b)")
                nc.vector.tensor_tensor(sq[:, :], pflat, pflat, op=ALU.mult)
                # column sums via ones-matmul into a 2-bank tile
                psq = ps_c.tile([128, 2, 512], F32, tag="pc",
                                name=f"psq_{c}")
                sqv = sq[:, :].rearrange("p (a b) -> p a b", a=ND)
                for h in range(2):
                    for d in range(ND):
                        nc.tensor.matmul(
                            psq[0:1, h, :500], ones_b[:, :],
                            sqv[:, d, h * 500:(h + 1) * 500],
                            start=(d == 0), stop=(d == ND - 1))
                # 1/||p|| straight into partition 0 of the broadcast buffer
                nc.scalar.activation(
                    invpb[0:1, c0:c1].rearrange("p (h x) -> p h x", h=2),
                    psq[0:1, :, :500], ACTF.Abs_reciprocal_sqrt)
                # broadcast to all 128 partitions (bf16 matmul) + copy out
                psb = ps_c.tile([128, 2, 512], F32, tag="pc",
                                name=f"psb_{c}")
                for h in range(2):
                    q0 = c0 + h * 500
                    nc.tensor.matmul(psb[:, h, :500], ones_r[:, :],
                                     invpb[0:1, q0:q0 + 500],
                                     start=True, stop=True)
                nc.scalar.copy(
                    invpb[:, c0:c1].rearrange("p (h x) -> p h x", h=2),
                    psb[:, :, :500])
                # in-place normalize: pn = pn * invp  (bf16, per d-tile)
                for d in range(ND):
                    nc.vector.tensor_tensor(pn[:, c, d, :],
                                            pn[:, c, d, :],
                                            invpb[:, c0:c1], op=ALU.mult)

            # ---- main loop -------------------------------------------------
            for i in range(NB):
                rs = small_pool.tile([128, NG], F32, tag="rs", bufs=NI,
                                     name=f"rs_{i}")
                dq = dq_pool.tile([128, C], BF16, tag="dq", name=f"dq_{i}")
                for g in range(NG):
                    gemm_group(i, g, dq, rs)
                finish_tile(i, dq, rs)

    nc.compile()
    return nc


_CACHE = {}


def _get_nc():
    if "nc" not in _CACHE:
        _CACHE["nc"] = build_nc()
    return _CACHE["nc"]


def make_in_maps(features, prototypes, distance_scale, temperature):
    f = np.asarray(features, dtype=np.float32)
    # negated so ACT's positive per-partition scale yields 1 - cos
    fneg = (-f).astype(ml_dtypes.bfloat16)
    pt = np.ascontiguousarray(
        np.asarray(prototypes, dtype=np.float32).T).astype(ml_dtypes.bfloat16)
    s = np.array([[np.float32(np.asarray(distance_scale).reshape(-1)[0]),
                   np.float32(np.asarray(temperature).reshape(-1)[0])]],
                 dtype=np.float32)
    in_maps = []
    for i in range(N_CORES):
        fi = fneg[i * BPC:(i + 1) * BPC]
        in_maps.append({
            "ft": np.ascontiguousarray(fi.T),
            "fb": np.ascontiguousarray(fi),
            "pt": pt,
            "s": s,
        })
    return in_maps


def run(features, prototypes, distance_scale, temperature, **kwargs):
    nc = _get_nc()
    in_maps = make_in_maps(features, prototypes, distance_scale, temperature)
    res = run_bass_kernel_spmd(nc, in_maps, core_ids=list(range(N_CORES)),
                               **kwargs)
    out = np.concatenate(
        [np.asarray(res.results[i]["o"]) for i in range(N_CORES)],
        axis=0).astype(np.float32)
    return out, res


def kernel(features, prototypes, distance_scale, temperature):
    out, _ = run(features, prototypes, distance_scale, temperature)
    return out
